# revision 7
# baseline (speedup 1.0000x reference)
"""CSWin self-attention Trainium2 kernel.

Sharding: data-parallel over batch B=8 across 8 cores (1 image per core).
Per-core pipeline (image = 128x128 spatial, C=256):
  A) LayerNorm (gamma/beta folded into Wqkv on host) + PE-transpose to
     channel-major y^T  [2 x [128ch, 16384tok] bf16]; x loaded in
     [128, 512] double-token-tile DMAs.
  B) Per direction (horizontal / vertical), per stripe (64 stripes of
     2 rows/cols = seq 256, 4 heads x head_dim 32):
       qkv matmuls (q^T,k^T ch-major; v token-major),
       S^T = k^T.T @ q^T row-tiled 4 heads (K=32),
       exp on ScalarE (scale folded),
       O^T = V.T @ exp(S^T) col-tiled 4 heads (M=32) into psum cols 0:256,
       D   = ones.T @ exp(S^T) col-tiled 4 heads into psum cols 256:512
       (per-q softmax denominators, broadcast across the 32 head rows),
       normalize straight from PSUM: drec=1/D, hdst = O^T * drec.
  C) Projection matmul (h^T stripe-seq layout as weights) + bias +
     residual, two token tiles per PSUM bank / DMA.
"""

import math
from contextlib import ExitStack

import numpy as np
import ml_dtypes

import concourse.bass as bass
import concourse.bacc as bacc
import concourse.mybir as mybir
import concourse.tile as tile
from concourse.bass_utils import run_bass_kernel_spmd

F32 = mybir.dt.float32
BF16 = mybir.dt.bfloat16
AF = mybir.ActivationFunctionType
ALU = mybir.AluOpType

B = 8
HH = 128
WW = 128
C = 256
T = HH * WW         # 16384 tokens
NT = T // 128       # 128 token tiles
NS = 64             # stripes per direction
SEQ = 256           # stripe seq len (2 * 128)
NHD = 4             # heads per direction
HD = 32
SCALE = HD ** -0.5
EPS = 1e-5


def build_nc(has_qbias: bool, has_pbias: bool) -> bass.Bass:
    nc = bacc.Bacc("TRN2", target_bir_lowering=False, debug=False)
    x_h = nc.dram_tensor("x", [T, C], F32, kind="ExternalInput")
    wqkv_h = nc.dram_tensor("wqkv", [2, 128, 768], BF16, kind="ExternalInput")
    wproj_h = nc.dram_tensor("wproj", [2, 128, 256], BF16, kind="ExternalInput")
    bqkv_h = nc.dram_tensor("bqkv", [1, 768], BF16, kind="ExternalInput")
    bproj_h = nc.dram_tensor("bproj", [1, 256], BF16, kind="ExternalInput")
    ident_h = nc.dram_tensor("ident", [128, 128], BF16, kind="ExternalInput")
    out_h = nc.dram_tensor("out", [T, C], F32, kind="ExternalOutput")

    with tile.TileContext(nc) as tc, tc.tile_pool(name="persist", bufs=1) as pp:
        # ---------------- persistent SBUF ----------------
        ytA = pp.tile([128, T], BF16, name="ytA", tag="ytA")
        ytB = pp.tile([128, T], BF16, name="ytB", tag="ytB")
        hHt = pp.tile([128, T], BF16, name="hHt", tag="hHt")
        hVt = pp.tile([128, T], BF16, name="hVt", tag="hVt")
        wqkv = pp.tile([128, 2 * 768], BF16, name="wqkv", tag="wqkv")
        wproj = pp.tile([128, 2 * 256], BF16, name="wproj", tag="wproj")
        brow = pp.tile([1, 768], BF16, name="brow", tag="brow")
        bprow = pp.tile([1, 256], BF16, name="bprow", tag="bprow")
        ones = pp.tile([1, 256], BF16, name="ones", tag="ones")
        ones32 = pp.tile([128, 32], BF16, name="ones32", tag="ones32")
        ident = pp.tile([128, 128], BF16, name="ident", tag="ident")

        nc.sync.dma_start(out=wqkv[:, 0:768], in_=wqkv_h[0])
        nc.sync.dma_start(out=wqkv[:, 768:1536], in_=wqkv_h[1])
        nc.sync.dma_start(out=wproj[:, 0:256], in_=wproj_h[0])
        nc.sync.dma_start(out=wproj[:, 256:512], in_=wproj_h[1])
        if has_qbias:
            nc.sync.dma_start(out=brow[:], in_=bqkv_h[:])
        if has_pbias:
            nc.sync.dma_start(out=bprow[:], in_=bproj_h[:])
        nc.vector.memset(ones[:], 1.0)
        nc.vector.memset(ones32[:], 1.0)
        nc.sync.dma_start(out=ident[:], in_=ident_h[:, :])

        # ---------------- phase A: LN + transpose ----------------
        with (
            tc.tile_pool(name="xa", bufs=3) as xa_pool,
            tc.tile_pool(name="ya", bufs=3) as ya_pool,
            tc.tile_pool(name="sa", bufs=2) as sa_pool,
            tc.tile_pool(name="tp", bufs=2, space="PSUM") as tp_pool,
        ):
            for j in range(NT // 2):
                xt = xa_pool.tile([128, 2, 256], F32, tag="xt")
                xin = x_h[256 * j:256 * j + 256, :].rearrange(
                    "(two p) c -> p two c", two=2)
                nc.sync.dma_start(out=xt[:], in_=xin)
                st6 = sa_pool.tile([128, 2, 6], F32, tag="st6")
                mv = sa_pool.tile([128, 2, 2], F32, tag="mv")
                rs = sa_pool.tile([128, 2, 3], F32, tag="rs")
                nmr = sa_pool.tile([128, 2, 1], F32, tag="nmr")
                yt_ = ya_pool.tile([128, 512], BF16, tag="yt")
                for b in range(2):
                    sl = slice(b * 256, b * 256 + 256)
                    nc.vector.bn_stats(st6[:, b], xt[:, b, :])
                    nc.vector.bn_aggr(mv[:, b], st6[:, b])
                    # rs: [var+eps, sqrt(var+eps), rstd]
                    nc.vector.tensor_scalar_add(rs[:, b, 0:1], mv[:, b, 1:2], EPS)
                    nc.scalar.activation(rs[:, b, 1:2], rs[:, b, 0:1], AF.Sqrt)
                    nc.vector.reciprocal(rs[:, b, 2:3], rs[:, b, 1:2])
                    # nmr = -mean * rstd; apply LN on ScalarE: y = x*rstd + nmr
                    nc.vector.tensor_scalar(
                        nmr[:, b, 0:1], mv[:, b, 0:1], rs[:, b, 2:3], -1.0,
                        ALU.mult, ALU.mult,
                    )
                    nc.scalar.activation(
                        yt_[:, sl], xt[:, b, :], AF.Identity,
                        bias=nmr[:, b, 0:1], scale=rs[:, b, 2:3],
                    )
                tp = tp_pool.tile([128, 512], BF16, tag="tp")
                for b in range(2):
                    nc.tensor.transpose(
                        tp[:, b * 256:b * 256 + 128],
                        yt_[:, b * 256:b * 256 + 128], ident[:])
                    nc.tensor.transpose(
                        tp[:, b * 256 + 128:b * 256 + 256],
                        yt_[:, b * 256 + 128:b * 256 + 256], ident[:])
                tpv = tp[:].rearrange("p (b h c) -> p b h c", b=2, h=2)
                nc.vector.tensor_copy(ytA[:, 256 * j:256 * j + 256], tpv[:, :, 0, :])
                nc.scalar.copy(ytB[:, 256 * j:256 * j + 256], tpv[:, :, 1, :])

        # stripe-sliced channel-major views of y^T
        ytAh = ytA[:].rearrange("p (h w) -> p h w", h=HH)
        ytBh = ytB[:].rearrange("p (h w) -> p h w", h=HH)
        ytAv = ytA[:].rearrange("p (h w) -> p w h", h=HH)
        ytBv = ytB[:].rearrange("p (h w) -> p w h", h=HH)

        # ---------------- phase B: attention ----------------
        with (
            tc.tile_pool(name="qkps", bufs=2, space="PSUM") as qk_pool,
            tc.tile_pool(name="vps", bufs=1, space="PSUM") as v_pool,
            tc.tile_pool(name="sps", bufs=1, space="PSUM") as s_pool,
            tc.tile_pool(name="odps", bufs=1, space="PSUM") as od_pool,
            tc.tile_pool(name="qksb", bufs=3) as qksb_pool,
            tc.tile_pool(name="vsb", bufs=3) as vsb_pool,
            tc.tile_pool(name="esb", bufs=2) as esb_pool,
            tc.tile_pool(name="rsb", bufs=3) as r_pool,
        ):
            def emit_od(prev):
                """O/D matmuls + normalize for a finished stripe (1 behind)."""
                e_sb, v_sb, hdst, g = prev
                od_ps = od_pool.tile([128, 512], F32, tag="odps")
                for sc in range(2):
                    for h in range(NHD):
                        nc.tensor.matmul(
                            od_ps[32 * h:32 * h + 32, 0:256],
                            lhsT=v_sb[:, sc, h, :],
                            rhs=e_sb[:, h * 512 + sc * 256:h * 512 + sc * 256 + 256],
                            start=sc == 0, stop=sc == 1,
                            tile_position=(0, 32 * h))
                for sc in range(2):
                    for h in range(NHD):
                        nc.tensor.matmul(
                            od_ps[32 * h:32 * h + 32, 256:512],
                            lhsT=ones32[:],
                            rhs=e_sb[:, h * 512 + sc * 256:h * 512 + sc * 256 + 256],
                            start=sc == 0, stop=sc == 1,
                            tile_position=(0, 32 * h))
                drec = r_pool.tile([128, 256], F32, tag="drec")
                nc.vector.reciprocal_approx_fast(drec[:], od_ps[:, 256:512])
                nc.vector.tensor_mul(
                    hdst[:, g * 256:(g + 1) * 256], od_ps[:, 0:256], drec[:])

            prev = None
            for di in range(2):
                horiz = di == 0
                qoff = 0 if horiz else 128
                hdst = hHt if horiz else hVt
                yviews = (ytAh, ytBh) if horiz else (ytAv, ytBv)
                for g in range(NS):
                    # rhs views: [128, 2, 128] seq-ordered stripe slice
                    rview = [yv[:, 2 * g:2 * g + 2, :] for yv in yviews]
                    # ---- qkv ----
                    qk_ps = qk_pool.tile([128, 512], F32, tag="qkps")
                    v_ps = v_pool.tile([128, 256], F32, tag="vps")
                    for kc in range(2):
                        wof = kc * 768
                        nc.tensor.matmul(
                            qk_ps[:, 0:256], lhsT=wqkv[:, wof + qoff:wof + qoff + 128],
                            rhs=rview[kc], start=kc == 0, stop=kc == 1 and not has_qbias)
                        nc.tensor.matmul(
                            qk_ps[:, 256:512], lhsT=wqkv[:, wof + 256 + qoff:wof + 384 + qoff],
                            rhs=rview[kc], start=kc == 0, stop=kc == 1 and not has_qbias)
                        for sc in range(2):
                            nc.tensor.matmul(
                                v_ps[:, sc * 128:sc * 128 + 128],
                                lhsT=rview[kc][:, sc, :],
                                rhs=wqkv[:, wof + 512 + qoff:wof + 640 + qoff],
                                start=kc == 0, stop=kc == 1 and not has_qbias)
                    if has_qbias:
                        nc.tensor.matmul(
                            qk_ps[:, 0:256], lhsT=brow[:, qoff:qoff + 128],
                            rhs=ones[:, 0:256], start=False, stop=True)
                        nc.tensor.matmul(
                            qk_ps[:, 256:512], lhsT=brow[:, 256 + qoff:384 + qoff],
                            rhs=ones[:, 0:256], start=False, stop=True)
                        for sc in range(2):
                            nc.tensor.matmul(
                                v_ps[:, sc * 128:sc * 128 + 128],
                                lhsT=ones[:, 0:128],
                                rhs=brow[:, 512 + qoff:640 + qoff],
                                start=False, stop=True)
                    qk_sb = qksb_pool.tile([128, 512], BF16, tag="qksb")
                    nc.vector.tensor_copy(qk_sb[:], qk_ps[:])
                    v_sb = vsb_pool.tile([128, 2, 4, 32], BF16, tag="vsb")
                    nc.vector.tensor_copy(
                        v_sb[:],
                        v_ps[:].rearrange("p (s h d) -> p s h d", s=2, h=4),
                    )
                    # ---- O/D + normalize for the PREVIOUS stripe (keeps the
                    #      PE busy while the DVE casts land, and off the
                    #      critical path of this stripe's exp) ----
                    if prev is not None:
                        emit_od(prev)
                    # ---- S^T (row-tiled 4 heads, K=32) ----
                    s_ps = s_pool.tile([128, 2048], F32, tag="sps")
                    for h in range(NHD):
                        for sc in range(2):
                            nc.tensor.matmul(
                                s_ps[:, h * 512 + sc * 256:h * 512 + sc * 256 + 256],
                                lhsT=qk_sb[32 * h:32 * h + 32, 256 + sc * 128:384 + sc * 128],
                                rhs=qk_sb[32 * h:32 * h + 32, 0:256],
                                start=True, stop=True,
                                tile_position=(32 * h, 0))
                    # ---- exp ----
                    e_sb = esb_pool.tile([128, 2048], BF16, tag="esb")
                    nc.scalar.activation(e_sb[:], s_ps[:], AF.Exp, scale=SCALE)
                    prev = (e_sb, v_sb, hdst, g)

            emit_od(prev)

        # ---------------- phase C: projection + residual ----------------
        hVv = hVt[:].rearrange("p (g j h) -> p g j h", g=NS, j=2)
        with (
            tc.tile_pool(name="pps", bufs=2, space="PSUM") as p_pool,
            tc.tile_pool(name="xr", bufs=3) as xr_pool,
            tc.tile_pool(name="po", bufs=3) as po_pool,
        ):
            for j in range(NT // 2):
                p_ps = p_pool.tile([128, 512], F32, tag="pps")
                for ii in range(2):
                    i = 2 * j + ii
                    csl = slice(ii * 256, ii * 256 + 256)
                    nc.tensor.matmul(
                        p_ps[:, csl], lhsT=hHt[:, i * 128:(i + 1) * 128],
                        rhs=wproj[:, 0:256], start=True, stop=False)
                    nc.tensor.matmul(
                        p_ps[:, csl], lhsT=hVv[:, :, :, i],
                        rhs=wproj[:, 256:512], start=False, stop=not has_pbias)
                    if has_pbias:
                        nc.tensor.matmul(
                            p_ps[:, csl], lhsT=ones[:, 0:128], rhs=bprow[:],
                            start=False, stop=True)
                xr = xr_pool.tile([128, 512], F32, tag="xr")
                xin = x_h[256 * j:256 * j + 256, :].rearrange(
                    "(two p) c -> p two c", two=2)
                nc.sync.dma_start(out=xr[:].rearrange("p (two c) -> p two c", two=2),
                                  in_=xin)
                po = po_pool.tile([128, 512], F32, tag="po")
                nc.vector.tensor_add(po[:], p_ps[:], xr[:])
                oout = out_h[256 * j:256 * j + 256, :].rearrange(
                    "(two p) c -> p two c", two=2)
                nc.sync.dma_start(out=oout,
                                  in_=po[:].rearrange("p (two c) -> p two c", two=2))

    return nc


_NC_CACHE = {}


def _get_nc(has_qbias, has_pbias):
    key = (has_qbias, has_pbias)
    if key not in _NC_CACHE:
        nc = build_nc(has_qbias, has_pbias)
        nc.finalize()
        _NC_CACHE[key] = nc
    return _NC_CACHE[key]


def kernel(x, Wqkv, bqkv, Wproj, bproj, gamma, beta, _trace=False, _tmpdir=None):
    x = np.asarray(x, np.float32)
    Wqkv = np.asarray(Wqkv, np.float32)
    bqkv = np.asarray(bqkv, np.float32)
    Wproj = np.asarray(Wproj, np.float32)
    bproj = np.asarray(bproj, np.float32)
    gamma = np.asarray(gamma, np.float32)
    beta = np.asarray(beta, np.float32)

    Wg = gamma[:, None] * Wqkv                      # fold LN affine scale
    bq = beta @ Wqkv + bqkv                         # fold LN affine shift
    has_qbias = bool(np.any(bq != 0.0))
    has_pbias = bool(np.any(bproj != 0.0))

    bf = ml_dtypes.bfloat16
    wqkv_np = np.ascontiguousarray(Wg.reshape(2, 128, 768)).astype(bf)
    wproj_np = np.ascontiguousarray(Wproj.reshape(2, 128, 256)).astype(bf)
    bq_np = bq.reshape(1, 768).astype(bf)
    bp_np = bproj.reshape(1, 256).astype(bf)

    nc = _get_nc(has_qbias, has_pbias)
    in_maps = []
    for b in range(B):
        in_maps.append({
            "x": np.ascontiguousarray(x[b].reshape(T, C)),
            "wqkv": wqkv_np, "wproj": wproj_np,
            "bqkv": bq_np, "bproj": bp_np,
            "ident": np.eye(128, dtype=np.float32).astype(bf),
        })
    res = run_bass_kernel_spmd(nc, in_maps, list(range(B)), trace=_trace,
                               tmpdir=_tmpdir)
    out = np.stack([np.asarray(res.results[b]["out"]).reshape(HH, WW, C)
                    for b in range(B)])
    if _trace:
        return out.astype(np.float32), res
    return out.astype(np.float32)


# revision 8
# speedup vs baseline: 1.3290x; 1.3290x over previous
"""CSWin self-attention Trainium2 kernel.

Sharding: data-parallel over batch B=8 across 8 cores (1 image per core).
Per-core pipeline (image = 128x128 spatial, C=256):
  A) LayerNorm (gamma/beta folded into Wqkv on host) + PE-transpose to
     channel-major y^T  [2 x [128ch, 16384tok] bf16]; x loaded in
     [128, 512] double-token-tile DMAs.
  B) Per direction (horizontal / vertical), per stripe (64 stripes of
     2 rows/cols = seq 256, 4 heads x head_dim 32):
       qkv matmuls (q^T,k^T ch-major; v token-major),
       S^T = k^T.T @ q^T row-tiled 4 heads (K=32),
       exp on ScalarE (scale folded),
       O^T = V.T @ exp(S^T) col-tiled 4 heads (M=32) into psum cols 0:256,
       D   = ones.T @ exp(S^T) col-tiled 4 heads into psum cols 256:512
       (per-q softmax denominators, broadcast across the 32 head rows),
       normalize straight from PSUM: drec=1/D, hdst = O^T * drec.
  C) Projection matmul (h^T stripe-seq layout as weights) + bias +
     residual, two token tiles per PSUM bank / DMA.
"""

import math
from contextlib import ExitStack

import numpy as np
import ml_dtypes

import concourse.bass as bass
import concourse.bacc as bacc
import concourse.mybir as mybir
import concourse.tile as tile
from concourse.bass_utils import run_bass_kernel_spmd

F32 = mybir.dt.float32
BF16 = mybir.dt.bfloat16
AF = mybir.ActivationFunctionType
ALU = mybir.AluOpType

B = 8
HH = 128
WW = 128
C = 256
T = HH * WW         # 16384 tokens
NT = T // 128       # 128 token tiles
NS = 64             # stripes per direction
SEQ = 256           # stripe seq len (2 * 128)
NHD = 4             # heads per direction
HD = 32
SCALE = HD ** -0.5
EPS = 1e-5


def build_nc(has_qbias: bool, has_pbias: bool) -> bass.Bass:
    nc = bacc.Bacc("TRN2", target_bir_lowering=False, debug=False)
    x_h = nc.dram_tensor("x", [T, C], F32, kind="ExternalInput")
    wqkv_h = nc.dram_tensor("wqkv", [2, 128, 768], BF16, kind="ExternalInput")
    wproj_h = nc.dram_tensor("wproj", [2, 128, 256], BF16, kind="ExternalInput")
    bqkv_h = nc.dram_tensor("bqkv", [1, 768], BF16, kind="ExternalInput")
    bproj_h = nc.dram_tensor("bproj", [1, 256], BF16, kind="ExternalInput")
    ident_h = nc.dram_tensor("ident", [128, 128], BF16, kind="ExternalInput")
    out_h = nc.dram_tensor("out", [T, C], F32, kind="ExternalOutput")

    with tile.TileContext(nc) as tc, tc.tile_pool(name="persist", bufs=1) as pp:
        # ---------------- persistent SBUF ----------------
        ytA = pp.tile([128, T], BF16, name="ytA", tag="ytA")
        ytB = pp.tile([128, T], BF16, name="ytB", tag="ytB")
        hHt = pp.tile([128, T], BF16, name="hHt", tag="hHt")
        hVt = pp.tile([128, T], BF16, name="hVt", tag="hVt")
        wqkv = pp.tile([128, 2 * 768], BF16, name="wqkv", tag="wqkv")
        wproj = pp.tile([128, 2 * 256], BF16, name="wproj", tag="wproj")
        brow = pp.tile([1, 768], BF16, name="brow", tag="brow")
        bprow = pp.tile([1, 256], BF16, name="bprow", tag="bprow")
        ones = pp.tile([1, 256], BF16, name="ones", tag="ones")
        ones32 = pp.tile([128, 32], BF16, name="ones32", tag="ones32")
        ident = pp.tile([128, 128], BF16, name="ident", tag="ident")

        nc.sync.dma_start(out=wqkv[:, 0:768], in_=wqkv_h[0])
        nc.sync.dma_start(out=wqkv[:, 768:1536], in_=wqkv_h[1])
        nc.sync.dma_start(out=wproj[:, 0:256], in_=wproj_h[0])
        nc.sync.dma_start(out=wproj[:, 256:512], in_=wproj_h[1])
        if has_qbias:
            nc.sync.dma_start(out=brow[:], in_=bqkv_h[:])
        if has_pbias:
            nc.sync.dma_start(out=bprow[:], in_=bproj_h[:])
        nc.vector.memset(ones[:], 1.0)
        nc.vector.memset(ones32[:], 1.0)
        nc.sync.dma_start(out=ident[:], in_=ident_h[:, :])

        # ---------------- phase A: LN + transpose ----------------
        with (
            tc.tile_pool(name="xa", bufs=3) as xa_pool,
            tc.tile_pool(name="ya", bufs=3) as ya_pool,
            tc.tile_pool(name="sa", bufs=2) as sa_pool,
            tc.tile_pool(name="tp", bufs=2, space="PSUM") as tp_pool,
        ):
            for j in range(NT // 2):
                xt = xa_pool.tile([128, 2, 256], F32, tag="xt")
                xin = x_h[256 * j:256 * j + 256, :].rearrange(
                    "(two p) c -> p two c", two=2)
                nc.sync.dma_start(out=xt[:], in_=xin)
                st6 = sa_pool.tile([128, 2, 6], F32, tag="st6")
                mv = sa_pool.tile([128, 2, 2], F32, tag="mv")
                rs = sa_pool.tile([128, 2, 3], F32, tag="rs")
                yt_ = ya_pool.tile([128, 512], BF16, tag="yt")
                for b in range(2):
                    sl = slice(b * 256, b * 256 + 256)
                    nc.vector.bn_stats(st6[:, b], xt[:, b, :])
                    nc.vector.bn_aggr(mv[:, b], st6[:, b])
                    # rs: [var+eps, sqrt(var+eps), rstd]
                    nc.vector.tensor_scalar_add(rs[:, b, 0:1], mv[:, b, 1:2], EPS)
                    nc.scalar.activation(rs[:, b, 1:2], rs[:, b, 0:1], AF.Sqrt)
                    nc.vector.reciprocal(rs[:, b, 2:3], rs[:, b, 1:2])
                    nc.vector.tensor_scalar(
                        yt_[:, sl], xt[:, b, :], mv[:, b, 0:1], rs[:, b, 2:3],
                        ALU.subtract, ALU.mult,
                    )
                tp = tp_pool.tile([128, 512], BF16, tag="tp")
                for b in range(2):
                    nc.tensor.transpose(
                        tp[:, b * 256:b * 256 + 128],
                        yt_[:, b * 256:b * 256 + 128], ident[:])
                    nc.tensor.transpose(
                        tp[:, b * 256 + 128:b * 256 + 256],
                        yt_[:, b * 256 + 128:b * 256 + 256], ident[:])
                tpv = tp[:].rearrange("p (b h c) -> p b h c", b=2, h=2)
                nc.vector.tensor_copy(ytA[:, 256 * j:256 * j + 256], tpv[:, :, 0, :])
                nc.scalar.copy(ytB[:, 256 * j:256 * j + 256], tpv[:, :, 1, :])

        # stripe-sliced channel-major views of y^T
        ytAh = ytA[:].rearrange("p (h w) -> p h w", h=HH)
        ytBh = ytB[:].rearrange("p (h w) -> p h w", h=HH)
        ytAv = ytA[:].rearrange("p (h w) -> p w h", h=HH)
        ytBv = ytB[:].rearrange("p (h w) -> p w h", h=HH)

        # ---------------- phase B: attention ----------------
        with (
            tc.tile_pool(name="qkps", bufs=2, space="PSUM") as qk_pool,
            tc.tile_pool(name="vps", bufs=1, space="PSUM") as v_pool,
            tc.tile_pool(name="sps", bufs=1, space="PSUM") as s_pool,
            tc.tile_pool(name="odps", bufs=1, space="PSUM") as od_pool,
            tc.tile_pool(name="qksb", bufs=3) as qksb_pool,
            tc.tile_pool(name="vsb", bufs=3) as vsb_pool,
            tc.tile_pool(name="esb", bufs=2) as esb_pool,
            tc.tile_pool(name="rsb", bufs=3) as r_pool,
        ):
            def emit_od(prev):
                """O/D matmuls + normalize for a finished stripe (1 behind)."""
                e_sb, v_sb, hdst, g = prev
                od_ps = od_pool.tile([128, 512], F32, tag="odps")
                for sc in range(2):
                    for h in range(NHD):
                        nc.tensor.matmul(
                            od_ps[32 * h:32 * h + 32, 0:256],
                            lhsT=v_sb[:, sc, h, :],
                            rhs=e_sb[:, h * 512 + sc * 256:h * 512 + sc * 256 + 256],
                            start=sc == 0, stop=sc == 1,
                            tile_position=(0, 32 * h))
                for sc in range(2):
                    for h in range(NHD):
                        nc.tensor.matmul(
                            od_ps[32 * h:32 * h + 32, 256:512],
                            lhsT=ones32[:],
                            rhs=e_sb[:, h * 512 + sc * 256:h * 512 + sc * 256 + 256],
                            start=sc == 0, stop=sc == 1,
                            tile_position=(0, 32 * h))
                drec = r_pool.tile([128, 256], F32, tag="drec")
                nc.vector.reciprocal_approx_fast(drec[:], od_ps[:, 256:512])
                nc.vector.tensor_mul(
                    hdst[:, g * 256:(g + 1) * 256], od_ps[:, 0:256], drec[:])

            prev = None
            for di in range(2):
                horiz = di == 0
                qoff = 0 if horiz else 128
                hdst = hHt if horiz else hVt
                yviews = (ytAh, ytBh) if horiz else (ytAv, ytBv)
                for g in range(NS):
                    # rhs views: [128, 2, 128] seq-ordered stripe slice
                    rview = [yv[:, 2 * g:2 * g + 2, :] for yv in yviews]
                    # ---- qkv ----
                    qk_ps = qk_pool.tile([128, 512], F32, tag="qkps")
                    v_ps = v_pool.tile([128, 256], F32, tag="vps")
                    for kc in range(2):
                        wof = kc * 768
                        nc.tensor.matmul(
                            qk_ps[:, 0:256], lhsT=wqkv[:, wof + qoff:wof + qoff + 128],
                            rhs=rview[kc], start=kc == 0, stop=kc == 1 and not has_qbias)
                        nc.tensor.matmul(
                            qk_ps[:, 256:512], lhsT=wqkv[:, wof + 256 + qoff:wof + 384 + qoff],
                            rhs=rview[kc], start=kc == 0, stop=kc == 1 and not has_qbias)
                        for sc in range(2):
                            nc.tensor.matmul(
                                v_ps[:, sc * 128:sc * 128 + 128],
                                lhsT=rview[kc][:, sc, :],
                                rhs=wqkv[:, wof + 512 + qoff:wof + 640 + qoff],
                                start=kc == 0, stop=kc == 1 and not has_qbias)
                    if has_qbias:
                        nc.tensor.matmul(
                            qk_ps[:, 0:256], lhsT=brow[:, qoff:qoff + 128],
                            rhs=ones[:, 0:256], start=False, stop=True)
                        nc.tensor.matmul(
                            qk_ps[:, 256:512], lhsT=brow[:, 256 + qoff:384 + qoff],
                            rhs=ones[:, 0:256], start=False, stop=True)
                        for sc in range(2):
                            nc.tensor.matmul(
                                v_ps[:, sc * 128:sc * 128 + 128],
                                lhsT=ones[:, 0:128],
                                rhs=brow[:, 512 + qoff:640 + qoff],
                                start=False, stop=True)
                    qk_sb = qksb_pool.tile([128, 512], BF16, tag="qksb")
                    nc.vector.tensor_copy(qk_sb[:], qk_ps[:])
                    v_sb = vsb_pool.tile([128, 2, 4, 32], BF16, tag="vsb")
                    nc.vector.tensor_copy(
                        v_sb[:],
                        v_ps[:].rearrange("p (s h d) -> p s h d", s=2, h=4),
                    )
                    # ---- S^T (row-tiled 4 heads, K=32) ----
                    s_ps = s_pool.tile([128, 2048], F32, tag="sps")
                    for h in range(NHD):
                        for sc in range(2):
                            nc.tensor.matmul(
                                s_ps[:, h * 512 + sc * 256:h * 512 + sc * 256 + 256],
                                lhsT=qk_sb[32 * h:32 * h + 32, 256 + sc * 128:384 + sc * 128],
                                rhs=qk_sb[32 * h:32 * h + 32, 0:256],
                                start=True, stop=True,
                                tile_position=(32 * h, 0))
                    # ---- O/D + normalize for the PREVIOUS stripe (off the
                    #      critical path of this stripe's exp) ----
                    if prev is not None:
                        emit_od(prev)
                    # ---- exp ----
                    e_sb = esb_pool.tile([128, 2048], BF16, tag="esb")
                    nc.scalar.activation(e_sb[:], s_ps[:], AF.Exp, scale=SCALE)
                    prev = (e_sb, v_sb, hdst, g)

            emit_od(prev)

        # ---------------- phase C: projection + residual ----------------
        hVv = hVt[:].rearrange("p (g j h) -> p g j h", g=NS, j=2)
        with (
            tc.tile_pool(name="pps", bufs=2, space="PSUM") as p_pool,
            tc.tile_pool(name="xr", bufs=3) as xr_pool,
            tc.tile_pool(name="po", bufs=3) as po_pool,
        ):
            for j in range(NT // 2):
                p_ps = p_pool.tile([128, 512], F32, tag="pps")
                for ii in range(2):
                    i = 2 * j + ii
                    csl = slice(ii * 256, ii * 256 + 256)
                    nc.tensor.matmul(
                        p_ps[:, csl], lhsT=hHt[:, i * 128:(i + 1) * 128],
                        rhs=wproj[:, 0:256], start=True, stop=False)
                    nc.tensor.matmul(
                        p_ps[:, csl], lhsT=hVv[:, :, :, i],
                        rhs=wproj[:, 256:512], start=False, stop=not has_pbias)
                    if has_pbias:
                        nc.tensor.matmul(
                            p_ps[:, csl], lhsT=ones[:, 0:128], rhs=bprow[:],
                            start=False, stop=True)
                xr = xr_pool.tile([128, 512], F32, tag="xr")
                xin = x_h[256 * j:256 * j + 256, :].rearrange(
                    "(two p) c -> p two c", two=2)
                nc.sync.dma_start(out=xr[:].rearrange("p (two c) -> p two c", two=2),
                                  in_=xin)
                po = po_pool.tile([128, 512], F32, tag="po")
                nc.vector.tensor_add(po[:], p_ps[:], xr[:])
                oout = out_h[256 * j:256 * j + 256, :].rearrange(
                    "(two p) c -> p two c", two=2)
                nc.sync.dma_start(out=oout,
                                  in_=po[:].rearrange("p (two c) -> p two c", two=2))

    return nc


_NC_CACHE = {}


def _get_nc(has_qbias, has_pbias):
    key = (has_qbias, has_pbias)
    if key not in _NC_CACHE:
        nc = build_nc(has_qbias, has_pbias)
        nc.finalize()
        _NC_CACHE[key] = nc
    return _NC_CACHE[key]


def kernel(x, Wqkv, bqkv, Wproj, bproj, gamma, beta, _trace=False, _tmpdir=None):
    x = np.asarray(x, np.float32)
    Wqkv = np.asarray(Wqkv, np.float32)
    bqkv = np.asarray(bqkv, np.float32)
    Wproj = np.asarray(Wproj, np.float32)
    bproj = np.asarray(bproj, np.float32)
    gamma = np.asarray(gamma, np.float32)
    beta = np.asarray(beta, np.float32)

    Wg = gamma[:, None] * Wqkv                      # fold LN affine scale
    bq = beta @ Wqkv + bqkv                         # fold LN affine shift
    has_qbias = bool(np.any(bq != 0.0))
    has_pbias = bool(np.any(bproj != 0.0))

    bf = ml_dtypes.bfloat16
    wqkv_np = np.ascontiguousarray(Wg.reshape(2, 128, 768)).astype(bf)
    wproj_np = np.ascontiguousarray(Wproj.reshape(2, 128, 256)).astype(bf)
    bq_np = bq.reshape(1, 768).astype(bf)
    bp_np = bproj.reshape(1, 256).astype(bf)

    nc = _get_nc(has_qbias, has_pbias)
    in_maps = []
    for b in range(B):
        in_maps.append({
            "x": np.ascontiguousarray(x[b].reshape(T, C)),
            "wqkv": wqkv_np, "wproj": wproj_np,
            "bqkv": bq_np, "bproj": bp_np,
            "ident": np.eye(128, dtype=np.float32).astype(bf),
        })
    res = run_bass_kernel_spmd(nc, in_maps, list(range(B)), trace=_trace,
                               tmpdir=_tmpdir)
    out = np.stack([np.asarray(res.results[b]["out"]).reshape(HH, WW, C)
                    for b in range(B)])
    if _trace:
        return out.astype(np.float32), res
    return out.astype(np.float32)
